# revision 2
# baseline (speedup 1.0000x reference)
"""Trainium2 Bass kernel for EnhancedMemoryEfficientAttention.

Sharding: 8 cores, core i owns spatial chunk i (2048 of 16384 positions)
for all 4 heads (each 2048x2048 attention block is independent per
(head, chunk)).  SE mean and GroupNorm stats are global -> computed from
one 66KB AllReduce of (s, C) where s = rowsum(attn_out), C = attn_out @
attn_out^T; SE gating and GN mu/var are derived analytically from (s, C)
so only a single collective sync is needed.

Key softmax trick: the module contracts the *unnormalized* axis
(out = V @ softmax(S)), so 1/Z folds into V^T columns ([128,32] scale per
strip) instead of a full [128,2048] pass, and exp's accum_out gives Z for
free on the ScalarE.
"""
import numpy as np

HEADS, DH, CHUNK, GROUPS, EPS = 4, 32, 2048, 8, 1e-5
INNER = HEADS * DH          # 128
HW = 16384
NCORES = 8
NSTRIP = CHUNK // 128       # 16 c-strips per block
SCALE = DH ** -0.5

_NC = None


def _build():
    from contextlib import ExitStack

    import concourse.bacc as bacc
    import concourse.tile as tile
    from concourse import mybir

    f32 = mybir.dt.float32
    bf16 = mybir.dt.bfloat16
    AF = mybir.ActivationFunctionType
    OP = mybir.AluOpType

    nc = bacc.Bacc("TRN2", target_bir_lowering=False, debug=False,
                   num_devices=NCORES)

    ins = {}

    def din(name, shape, dt):
        ins[name] = nc.dram_tensor(name, shape, dt, kind="ExternalInput").ap()

    din("x", [128, CHUNK], bf16)
    din("wq_t", [128, 128], bf16)
    din("wk_t", [128, 128], bf16)
    din("wv_t", [128, 128], bf16)
    din("out_w_t", [128, 128], f32)
    din("se_w1_t", [128, 32], f32)
    din("se_b1", [32, 1], f32)
    din("se_w2_t", [32, 128], f32)
    din("se_b2", [128, 1], f32)
    din("out_b", [128, 1], f32)
    din("out_b_2x", [128, 1], f32)
    din("out_b_sq", [128, 1], f32)
    din("gn_w", [128, 1], f32)
    din("gn_b", [128, 1], f32)
    din("ident", [128, 128], f32)
    din("gmask", [128, GROUPS], f32)
    din("gmask_t", [GROUPS, 128], f32)
    din("ones_hw", [128, 1], f32)

    out_d = nc.dram_tensor("out", [128, CHUNK], f32, kind="ExternalOutput").ap()

    with tile.TileContext(nc) as tc, ExitStack() as ctx:
        const = ctx.enter_context(tc.tile_pool(name="const", bufs=1))
        big = ctx.enter_context(tc.tile_pool(name="big", bufs=1))
        apool = ctx.enter_context(tc.tile_pool(name="apool", bufs=6))
        vpool = ctx.enter_context(tc.tile_pool(name="vpool", bufs=3))
        zpool = ctx.enter_context(tc.tile_pool(name="zpool", bufs=4))
        small = ctx.enter_context(tc.tile_pool(name="small", bufs=2))
        # PSUM budget: spsum 2 slots x [128,1024]f32 (2 banks) = 4 banks,
        # opsum 1 slot x [128,2048]f32 = 4 banks  -> 8 banks total.
        spsum = ctx.enter_context(tc.tile_pool(name="spsum", bufs=2, space="PSUM"))
        opsum = ctx.enter_context(tc.tile_pool(name="opsum", bufs=1, space="PSUM"))
        dram = ctx.enter_context(tc.tile_pool(name="dram", bufs=1, space="DRAM"))

        # ---- load constants / inputs to SBUF
        def cload(name, shape, dt):
            t = const.tile(shape, dt, tag=name)
            nc.sync.dma_start(out=t, in_=ins[name])
            return t

        xw = cload("x", [128, CHUNK], bf16)
        wq = cload("wq_t", [128, 128], bf16)
        wk = cload("wk_t", [128, 128], bf16)
        wv = cload("wv_t", [128, 128], bf16)
        out_w_t = cload("out_w_t", [128, 128], f32)
        se_w1_t = cload("se_w1_t", [128, 32], f32)
        se_b1 = cload("se_b1", [32, 1], f32)
        se_w2_t = cload("se_w2_t", [32, 128], f32)
        se_b2 = cload("se_b2", [128, 1], f32)
        out_b = cload("out_b", [128, 1], f32)
        out_b_2x = cload("out_b_2x", [128, 1], f32)
        out_b_sq = cload("out_b_sq", [128, 1], f32)
        gn_w = cload("gn_w", [128, 1], f32)
        gn_b = cload("gn_b", [128, 1], f32)
        ident = cload("ident", [128, 128], f32)
        gmask = cload("gmask", [128, GROUPS], f32)
        gmask_t = cload("gmask_t", [GROUPS, 128], f32)
        ones_hw = cload("ones_hw", [128, 1], f32)

        # ---- qkv: q = Wq @ x, k = Wk @ x (inner on partitions, bf16)
        q_sb = big.tile([128, CHUNK], bf16, tag="q")
        k_sb = big.tile([128, CHUNK], bf16, tag="k")
        vt_sb = big.tile([128, CHUNK], f32, tag="vt")  # [spatial, inner] per 128-chunk
        for half in range(2):
            for w_sb, dst in ((wq, q_sb), (wk, k_sb)):
                ps = spsum.tile([128, 1024], f32, tag="s")
                for j in range(2):
                    c0 = half * 1024 + j * 512
                    nc.tensor.matmul(ps[:, j * 512:(j + 1) * 512],
                                     lhsT=w_sb, rhs=xw[:, c0:c0 + 512],
                                     start=True, stop=True)
                nc.vector.tensor_copy(
                    out=dst[:, half * 1024:(half + 1) * 1024], in_=ps)
        # v^T directly: vT[p, d] = sum_cin x[cin, p] * WvT[cin, d]
        for ci in range(NSTRIP):
            ps = spsum.tile([128, 128], f32, tag="s")
            nc.tensor.matmul(ps, lhsT=xw[:, ci * 128:(ci + 1) * 128], rhs=wv,
                             start=True, stop=True)
            nc.vector.tensor_copy(out=vt_sb[:, ci * 128:(ci + 1) * 128], in_=ps)

        # ---- attention main loop
        o_ps = opsum.tile([128, CHUNK], f32, tag="o")
        out_sb = big.tile([128, CHUNK], f32, tag="outsb")
        s_part = small.tile([128, 1], f32, tag="spart")
        for h in range(HEADS):
            hb = slice(32 * h, 32 * h + 32)
            for ci in range(NSTRIP):
                a_tiles = []
                zs = []
                for eh in range(2):
                    s_ps = spsum.tile([128, 1024], f32, tag="s")
                    for j in range(2):
                        e0 = eh * 1024 + j * 512
                        nc.tensor.matmul(
                            s_ps[:, j * 512:(j + 1) * 512],
                            lhsT=q_sb[hb, ci * 128:(ci + 1) * 128],
                            rhs=k_sb[hb, e0:e0 + 512],
                            start=True, stop=True,
                            tile_position=(32 * h, 0))
                    a_t = apool.tile([128, 1024], bf16, tag="a")
                    z_t = zpool.tile([128, 1], f32, tag="z")
                    nc.scalar.activation(out=a_t, in_=s_ps, func=AF.Exp,
                                         scale=SCALE, accum_out=z_t)
                    a_tiles.append(a_t)
                    zs.append(z_t)
                zsum = zpool.tile([128, 1], f32, tag="zsum")
                nc.vector.tensor_add(out=zsum, in0=zs[0], in1=zs[1])
                rz = zpool.tile([128, 1], f32, tag="rz")
                nc.vector.reciprocal(out=rz, in_=zsum)
                # V''^T strip: V^T columns of head h scaled by 1/Z (per row)
                vtt = vpool.tile([128, 32], bf16, tag="vtt")
                nc.vector.tensor_scalar_mul(
                    out=vtt,
                    in0=vt_sb[:, ci * 128 + 32 * h: ci * 128 + 32 * h + 32],
                    scalar1=rz)
                for q4 in range(4):
                    a_t = a_tiles[q4 // 2]
                    nc.tensor.matmul(
                        o_ps[hb, q4 * 512:(q4 + 1) * 512],
                        lhsT=vtt,
                        rhs=a_t[:, (q4 % 2) * 512:(q4 % 2 + 1) * 512],
                        start=(ci == 0), stop=(ci == NSTRIP - 1),
                        tile_position=(0, 32 * h),
                        skip_group_check=True)
            # evacuate this head's O band; accum gives s partial rowsums
            nc.vector.tensor_scalar(
                out=out_sb[hb, :], in0=o_ps[hb, :],
                scalar1=1.0, scalar2=0.0, op0=OP.mult, op1=OP.add,
                accum_out=s_part[hb, :])

        # ---- C = out @ out^T (local partial) via PE transposes
        outT_sb = big.tile([128, CHUNK], f32, tag="outT")
        c_ps = opsum.tile([128, 128], f32, tag="o")  # reuses o slot after release
        for ci in range(NSTRIP):
            t_ps = spsum.tile([128, 128], f32, tag="s")
            nc.tensor.transpose(t_ps, out_sb[:, ci * 128:(ci + 1) * 128], ident)
            nc.vector.tensor_copy(out=outT_sb[:, ci * 128:(ci + 1) * 128],
                                  in_=t_ps)
            nc.tensor.matmul(c_ps,
                             lhsT=outT_sb[:, ci * 128:(ci + 1) * 128],
                             rhs=outT_sb[:, ci * 128:(ci + 1) * 128],
                             start=(ci == 0), stop=(ci == NSTRIP - 1),
                             skip_group_check=True)
        c_sb = small.tile([128, 128], f32, tag="csb")
        nc.vector.tensor_copy(out=c_sb, in_=c_ps)

        # ---- single AllReduce of [s | C]  (128 x 129 f32 = 66KB)
        cc_in = dram.tile([128, 129], f32, tag="ccin")
        cc_out = dram.tile([128, 129], f32, tag="ccout")
        nc.sync.dma_start(out=cc_in[:, 0:1], in_=s_part)
        nc.sync.dma_start(out=cc_in[:, 1:129], in_=c_sb)
        nc.gpsimd.collective_compute(
            "AllReduce", OP.add,
            replica_groups=[list(range(NCORES))],
            ins=[cc_in.opt()], outs=[cc_out.opt()])
        sc_sb = small.tile([128, 129], f32, tag="scsb")
        nc.sync.dma_start(out=sc_sb, in_=cc_out)

        # ---- SE gating (from global s)
        s_mean = small.tile([128, 1], f32, tag="smean")
        nc.vector.tensor_scalar_mul(out=s_mean, in0=sc_sb[:, 0:1],
                                    scalar1=1.0 / HW)
        z1_ps = spsum.tile([32, 1], f32, tag="s")
        nc.tensor.matmul(z1_ps, lhsT=se_w1_t, rhs=s_mean, start=True, stop=True)
        z1_sb = small.tile([32, 1], f32, tag="z1")
        nc.scalar.activation(out=z1_sb, in_=z1_ps, func=AF.Silu, bias=se_b1)
        g_ps = spsum.tile([128, 1], f32, tag="s")
        nc.tensor.matmul(g_ps, lhsT=se_w2_t, rhs=z1_sb, start=True, stop=True)
        g_sb = small.tile([128, 1], f32, tag="g")
        nc.scalar.activation(out=g_sb, in_=g_ps, func=AF.Sigmoid, bias=se_b2)

        # W'^T = out_w^T * g  (per-partition over cin=inner)
        wp_sb = small.tile([128, 128], f32, tag="wp")
        nc.vector.tensor_scalar_mul(out=wp_sb, in0=out_w_t, scalar1=g_sb)

        # ---- y = W' @ out (bias/GN affine folded later)
        y_ps = opsum.tile([128, CHUNK], f32, tag="o")
        for jc in range(4):
            nc.tensor.matmul(y_ps[:, jc * 512:(jc + 1) * 512],
                             lhsT=wp_sb, rhs=out_sb[:, jc * 512:(jc + 1) * 512],
                             start=True, stop=True)

        # ---- GroupNorm stats from (s, C):
        # E_p[y_o^2] = w'_o (C/HW) w'_o^T + 2 b_o (w'_o . s_mean) + b_o^2
        u_ps = spsum.tile([128, 128], f32, tag="s")
        nc.tensor.matmul(u_ps, lhsT=sc_sb[:, 1:129], rhs=wp_sb,
                         start=True, stop=True)
        u_sb = small.tile([128, 128], f32, tag="usb")
        nc.vector.tensor_copy(out=u_sb, in_=u_ps)
        v1_sb = small.tile([128, 128], f32, tag="v1")
        nc.vector.tensor_mul(out=v1_sb, in0=wp_sb, in1=u_sb)
        e2_ps = spsum.tile([128, 1], f32, tag="s")
        nc.tensor.matmul(e2_ps, lhsT=v1_sb, rhs=ones_hw, start=True, stop=True)
        mu_ps = spsum.tile([128, 1], f32, tag="s")
        nc.tensor.matmul(mu_ps, lhsT=wp_sb, rhs=s_mean, start=True, stop=True)

        stats = small.tile([128, 2], f32, tag="stats")
        # mu_chan = mu_core + out_b
        nc.vector.tensor_scalar_add(out=stats[:, 0:1], in0=mu_ps,
                                    scalar1=out_b)
        # e2_chan = e2_core + 2*out_b*mu_core + out_b^2
        t1 = small.tile([128, 1], f32, tag="t1")
        nc.vector.tensor_scalar_mul(out=t1, in0=mu_ps, scalar1=out_b_2x)
        t2 = small.tile([128, 1], f32, tag="t2")
        nc.vector.tensor_add(out=t2, in0=e2_ps, in1=t1)
        nc.vector.tensor_scalar_add(out=stats[:, 1:2], in0=t2,
                                    scalar1=out_b_sq)

        gm_ps = spsum.tile([GROUPS, 2], f32, tag="s")
        nc.tensor.matmul(gm_ps, lhsT=gmask, rhs=stats, start=True, stop=True)
        gm_sb = small.tile([GROUPS, 2], f32, tag="gm")
        nc.vector.tensor_copy(out=gm_sb, in_=gm_ps)
        m2 = small.tile([GROUPS, 1], f32, tag="m2")
        nc.vector.tensor_mul(out=m2, in0=gm_sb[:, 0:1], in1=gm_sb[:, 0:1])
        var = small.tile([GROUPS, 1], f32, tag="var")
        nc.vector.tensor_sub(out=var, in0=gm_sb[:, 1:2], in1=m2)
        eps_t = small.tile([GROUPS, 1], f32, tag="eps")
        nc.vector.memset(eps_t, EPS)
        sq = small.tile([GROUPS, 1], f32, tag="sq")
        nc.scalar.activation(out=sq, in_=var, func=AF.Sqrt, bias=eps_t)
        rsq = small.tile([GROUPS, 1], f32, tag="rsq")
        nc.vector.reciprocal(out=rsq, in_=sq)

        rm = small.tile([GROUPS, 2], f32, tag="rm")
        nc.vector.tensor_copy(out=rm[:, 0:1], in_=rsq)
        nc.vector.tensor_copy(out=rm[:, 1:2], in_=gm_sb[:, 0:1])
        bc_ps = spsum.tile([128, 2], f32, tag="s")
        nc.tensor.matmul(bc_ps, lhsT=gmask_t, rhs=rm, start=True, stop=True)
        bc_sb = small.tile([128, 2], f32, tag="bc")
        nc.vector.tensor_copy(out=bc_sb, in_=bc_ps)

        # alpha = rsq_o * gn_w ; beta = gn_b - alpha*(mu_o - out_b)
        alpha = small.tile([128, 1], f32, tag="alpha")
        nc.vector.tensor_scalar_mul(out=alpha, in0=bc_sb[:, 0:1], scalar1=gn_w)
        t3 = small.tile([128, 1], f32, tag="t3")
        nc.vector.scalar_tensor_tensor(out=t3, in0=bc_sb[:, 1:2],
                                       scalar=out_b, in1=alpha,
                                       op0=OP.subtract, op1=OP.mult)
        beta = small.tile([128, 1], f32, tag="beta")
        nc.vector.tensor_scalar(out=beta, in0=t3, scalar1=-1.0, scalar2=gn_b,
                                op0=OP.mult, op1=OP.add)

        # ---- final affine + store
        yn_sb = big.tile([128, CHUNK], f32, tag="yn")
        nc.scalar.activation(out=yn_sb, in_=y_ps, func=AF.Identity,
                             bias=beta, scale=alpha)
        nc.sync.dma_start(out=out_d, in_=yn_sb)

    nc.compile()
    return nc


def _get_nc():
    global _NC
    if _NC is None:
        _NC = _build()
    return _NC


def _host_inputs(x, w_qkv, se_w1, se_b1, se_w2, se_b2, out_w, out_b,
                 gn_w, gn_b):
    import ml_dtypes
    bf = ml_dtypes.bfloat16
    f32 = np.float32

    def c(a, dt=f32):
        return np.ascontiguousarray(np.asarray(a), dtype=dt)

    def col(v):
        return c(np.asarray(v).reshape(-1, 1))

    x2 = np.asarray(x, dtype=f32).reshape(INNER, HW)
    w_qkv = np.asarray(w_qkv, dtype=f32)

    gmask = np.zeros((128, GROUPS), f32)
    gmask[np.arange(128), np.arange(128) // (128 // GROUPS)] = 1.0 / (128 // GROUPS)
    gmask_t = np.zeros((GROUPS, 128), f32)
    gmask_t[np.arange(128) // (128 // GROUPS), np.arange(128)] = 1.0

    out_b = np.asarray(out_b, dtype=f32)
    shared = {
        "wq_t": c(w_qkv[0:128].T, bf),
        "wk_t": c(w_qkv[128:256].T, bf),
        "wv_t": c(w_qkv[256:384].T, bf),
        "out_w_t": c(np.asarray(out_w).T),
        "se_w1_t": c(np.asarray(se_w1).T),
        "se_b1": col(se_b1),
        "se_w2_t": c(np.asarray(se_w2).T),
        "se_b2": col(se_b2),
        "out_b": col(out_b),
        "out_b_2x": col(2.0 * out_b),
        "out_b_sq": col(out_b * out_b),
        "gn_w": col(gn_w),
        "gn_b": col(gn_b),
        "ident": np.eye(128, dtype=f32),
        "gmask": gmask,
        "gmask_t": gmask_t,
        "ones_hw": np.full((128, 1), 1.0 / HW, f32),
    }
    in_maps = []
    for i in range(NCORES):
        m = dict(shared)
        m["x"] = c(x2[:, i * CHUNK:(i + 1) * CHUNK], bf)
        in_maps.append(m)
    return in_maps


def kernel(x, w_qkv, se_w1, se_b1, se_w2, se_b2, out_w, out_b, gn_w, gn_b):
    from concourse.bass_utils import run_bass_kernel_spmd

    in_maps = _host_inputs(x, w_qkv, se_w1, se_b1, se_w2, se_b2, out_w,
                           out_b, gn_w, gn_b)
    res = run_bass_kernel_spmd(_get_nc(), in_maps, core_ids=list(range(NCORES)))
    y = np.concatenate([np.asarray(res.results[i]["out"], dtype=np.float32)
                        for i in range(NCORES)], axis=1)
    B, C, H, W = 1, 128, 128, 128
    return y.reshape(B, C, H, W)
